# revision 25
# baseline (speedup 1.0000x reference)
"""Trainium2 Bass kernel for nn_Attention_63513976373985.

Strategy: pure data-parallel over the batch dim B=64 across 8 NeuronCores
(8 batches per core, all params replicated, no collectives).

v4: batch-paired zs pipeline + warm start.
  - PE prewarm: 12 scratch matmuls bring the HAM clock-gate to K=8/8
    before the first real matmul (gpsimd memsets the scratch at ~6.3us,
    right after the fixed ~7us framework preamble).
  - Input DMAs split across both HWDGE queues so the d3(b0/b1) and
    d4/zc critical paths land ~10.6us / ~13.6us.
  - zs matmuls process batch PAIRS: rhs d3t_pair [128, 1024] (bf16
    moving-operand max) -> 2 MMs per (pair, h, ct) instead of 4, so
    LDWEIGHTS (93ns) fully hides under 426ns matmuls (baseline exposed
    ~95ns on a third of its N=512 matmuls).
  - Two atts accumulator banks (even/odd batch) so both pair batches'
    P-reduction quads interleave in the head loop.
  - ACT (tanh) is the binding engine (~104us busy): the tail chain
    (cast->comb->nmax->exp) of pair p is scheduled so exp lands in the
    ACT queue after pair p+1's first tanh, hiding the 4-engine latency.
  - us tanh stays 2 calls per (b, h): the per-ct bias zc (the d4@wbot
    term, constant over s) rides the ACT per-partition bias port; a
    single [128, 1024] call is impossible since 256 distinct bias
    values can't fit 128 partitions.

Everything fp16 (fp32 PSUM accumulate).
"""
import sys

if "/opt/trn_rl_repo" not in sys.path:
    sys.path.insert(0, "/opt/trn_rl_repo")

import numpy as np

H, F, C, S, B = 8, 512, 256, 512, 64
NCORES = 8
BLOC = B // NCORES  # 8
NPAIR = BLOC // 2   # 4
OUTF = 128

_CACHE = {}


def build_nc(debug=False, dbg_b=0):
    import concourse.bass as bass  # noqa: F401
    import concourse.mybir as mybir
    import concourse.tile as tile
    from concourse import bacc
    from contextlib import ExitStack

    f32 = mybir.dt.float32
    f16 = mybir.dt.float16
    AF = mybir.ActivationFunctionType
    ALU = mybir.AluOpType

    nc = bacc.Bacc("TRN2", target_bir_lowering=False, debug=False,
                   num_devices=NCORES)

    # ---- DRAM parameters (per-core shard shapes) ----
    xt_d = nc.dram_tensor("xt", [128, 4, BLOC, S], f16, kind="ExternalInput")
    w1_d = nc.dram_tensor("w1r", [128, 4, 2, 128], f16, kind="ExternalInput")
    wv_d = nc.dram_tensor("wvr", [128, 4, C], f16, kind="ExternalInput")
    wtop_d = nc.dram_tensor("wtopr", [128, H, 2, 2, 128], f16,
                            kind="ExternalInput")
    wbot_d = nc.dram_tensor("wbotr", [128, H, 2, 2, 128], f16,
                            kind="ExternalInput")
    wcc_d = nc.dram_tensor("wccr", [128, 2 * H, OUTF], f16,
                           kind="ExternalInput")
    # packed small consts: [0:8]=comb, [8:72]=pblk, [72:104]=d1t,
    # [104:112]=id8(rows 0:8), [112:120]=ones18(row 0), [120:248]=bcc(row 0)
    pk_d = nc.dram_tensor("packed", [128, 248], f16, kind="ExternalInput")
    b1c_d = nc.dram_tensor("b1c", [128, 2], f32, kind="ExternalInput")
    out_d = nc.dram_tensor("out", [BLOC, OUTF], f32, kind="ExternalOutput")

    with tile.TileContext(nc) as tc, ExitStack() as stk:
        const = stk.enter_context(tc.tile_pool(name="const", bufs=1))
        xtp = stk.enter_context(tc.tile_pool(name="xtp", bufs=1))
        d3p = stk.enter_context(tc.tile_pool(name="d3p", bufs=2))
        tvpool = stk.enter_context(tc.tile_pool(name="tvpool", bufs=4))
        usp = stk.enter_context(tc.tile_pool(name="usp", bufs=8))
        smallsb = stk.enter_context(tc.tile_pool(name="smallsb", bufs=2))
        vpool = stk.enter_context(tc.tile_pool(name="vpool", bufs=1))
        # PSUM budget (8 banks): pzs 2x[128,1024]f32=4, pmm [128,1024]f32=2,
        # patp 2x[128,512]f32=2.
        pmm = stk.enter_context(tc.tile_pool(name="pmm", bufs=1, space="PSUM"))
        pzs = stk.enter_context(tc.tile_pool(name="pzs", bufs=2, space="PSUM"))
        patp = stk.enter_context(tc.tile_pool(name="patp", bufs=2,
                                              space="PSUM"))

        xt_sb = xtp.tile([128, 4, BLOC, S], f16, tag="xt")
        w1_sb = const.tile([128, 4, 2, 128], f16, tag="w1")
        pk_sb = const.tile([128, 248], f16, tag="packed")
        b1c_sb = const.tile([128, 2], f32, tag="b1c")
        wv_sb = const.tile([128, 4, C], f16, tag="wv")
        wbot_sb = const.tile([128, H, 2, 2, 128], f16, tag="wbot")
        wtop_sb = const.tile([128, H, 2, 2, 128], f16, tag="wtop")
        wcc_sb = const.tile([128, 2 * H, OUTF], f16, tag="wcc")

        comb_sb = pk_sb[:, 0:8]

        def pblk_sl(idx):  # pblk[:, idx, :] from packed cols [8:72]
            return pk_sb[:, 8 + idx * 4:8 + idx * 4 + 4]

        def d1t_sl(k):  # d1t[:, k, :] from packed cols [72:104]
            return pk_sb[:, 72 + k * BLOC:72 + (k + 1) * BLOC]

        id8_sb = pk_sb[0:8, 104:112]
        ones18_sb = pk_sb[0:1, 112:120]
        bcc_sb = pk_sb[0:1, 120:248]

        # ---- DMA issue: 2 HWDGE queues. The framework preamble is ~7us,
        # first data moves ~8.6us. Critical chain: d3(pair0) 8 N=1024 MMs
        # (3.4us) after w1+xt[0:2] -> first zs -> first tanh. xt0/xt1 are
        # kf-split across both queues so the kf01 matmuls start ~1us
        # earlier; wbot/wtop stream in head order behind them.
        nc.sync.dma_start(out=w1_sb, in_=w1_d[:, :, :, :])
        nc.sync.dma_start(out=xt_sb[:, 0:2, 0, :], in_=xt_d[:, 0:2, 0, :])
        nc.sync.dma_start(out=xt_sb[:, 2:4, 0, :], in_=xt_d[:, 2:4, 0, :])
        nc.sync.dma_start(out=pk_sb, in_=pk_d[:, :])
        nc.sync.dma_start(out=b1c_sb, in_=b1c_d[:, :])
        nc.sync.dma_start(out=xt_sb[:, :, 2:4, :], in_=xt_d[:, :, 2:4, :])
        nc.sync.dma_start(out=xt_sb[:, :, 4:8, :], in_=xt_d[:, :, 4:8, :])
        nc.sync.dma_start(out=wcc_sb, in_=wcc_d[:, :, :])
        # scalar: xt1 halves -> wtop/wbot in head order -> wv
        nc.scalar.dma_start(out=xt_sb[:, 0:2, 1, :], in_=xt_d[:, 0:2, 1, :])
        nc.scalar.dma_start(out=xt_sb[:, 2:4, 1, :], in_=xt_d[:, 2:4, 1, :])
        nc.scalar.dma_start(out=wtop_sb[:, 0:2], in_=wtop_d[:, 0:2])
        nc.scalar.dma_start(out=wbot_sb[:, 0:2], in_=wbot_d[:, 0:2])
        nc.scalar.dma_start(out=wbot_sb[:, 2:4], in_=wbot_d[:, 2:4])
        nc.scalar.dma_start(out=wtop_sb[:, 2:4], in_=wtop_d[:, 2:4])
        nc.scalar.dma_start(out=wbot_sb[:, 4:8], in_=wbot_d[:, 4:8])
        nc.scalar.dma_start(out=wtop_sb[:, 4:8], in_=wtop_d[:, 4:8])
        nc.scalar.dma_start(out=wv_sb, in_=wv_d[:, :, :])

        # ---- PE prewarm: scratch matmuls from ~6.4us (gpsimd memset ends
        # ~6.3us) until real data lands (~11us) push HAM to K=8/8.
        warm_sb = const.tile([128, 512], f16, tag="warm")
        nc.gpsimd.memset(warm_sb, 0.0)
        pwarm = pmm.tile([128, 512], f32, tag="mm", name="prewarm")
        for _ in range(12):
            nc.tensor.matmul(pwarm, lhsT=warm_sb[:, 0:128], rhs=warm_sb,
                             start=True, stop=True)

        d4t_sb = const.tile([128, 2, BLOC], f16, tag="d4t")

        def emit_d4():
            # patp banks are unused until the first quads (~19us); pd4/pzc
            # borrow them so the pmm slot stays free for the split d3(0).
            pd4 = patp.tile([128, 2, BLOC], f32, tag="atts", name="pd4")
            for m in range(2):
                for k in range(4):
                    nc.tensor.matmul(pd4[:, m, :], lhsT=w1_sb[:, k, m, :],
                                     rhs=d1t_sl(k),
                                     start=(k == 0), stop=(k == 3))
            for m in range(2):
                nc.scalar.activation(d4t_sb[:, m, :], pd4[:, m, :], AF.Relu,
                                     bias=b1c_sb[:, m:m + 1])

        v_sb = vpool.tile([128, 2, H, BLOC], f16)  # [c-in-half, ch, h, b]
        zc_sb = const.tile([128, 2, H, BLOC], f32, tag="zc")

        def emit_zc(h0, h1):
            # per-head-group so zc[h<2] is ready as soon as wbot[0:2] lands
            pzc = patp.tile([128, 2, h1 - h0, BLOC], f32, tag="atts",
                            name=f"pzc{h0}")
            for ct in range(2):
                for h in range(h0, h1):
                    for ks in range(2):
                        nc.tensor.matmul(pzc[:, ct, h - h0, :],
                                         lhsT=wbot_sb[:, h, ks, ct, :],
                                         rhs=d4t_sb[:, ks, :],
                                         start=(ks == 0), stop=(ks == 1))
            nc.vector.tensor_copy(out=zc_sb[:, :, h0:h1, :], in_=pzc)

        # ---- pipeline state ----
        d3tps = [None] * NPAIR   # d3t_pair [128, 2, 1024]
        tvs = [None] * BLOC
        uss = {}
        pats = [None] * BLOC
        attsgs = [None] * BLOC
        pat2s = [None] * BLOC
        nmaxs = [None] * BLOC
        escs = [None] * BLOC
        zinvs = [None] * BLOC
        scts = [None] * BLOC
        vssbs = [None] * BLOC

        d30_state = {}

        def emit_d3_half(half):
            # pair-0 startup: per (batch-half, m) N=512 chains so b0's
            # d3t halves are ready ~1.5us before the full pair tile.
            # Two concurrent 2-bank accumulators: m0 from pmm, m1 borrows
            # a pzs buffer (free until the first zs tile).
            if half == 0:
                d30_state["d3t"] = d3p.tile([128, 2, 2 * S], f16,
                                            tag="d3t", name="d3t0")
                d30_state["pm"] = [
                    pmm.tile([128, 2 * S], f32, tag="mm", name="pmd3_0_0"),
                    pzs.tile([128, 2 * S], f32, tag="zs", name="pmd3_0_1")]
                d3tps[0] = d30_state["d3t"]
            d3t, pmd3s = d30_state["d3t"], d30_state["pm"]
            sl = slice(half * S, (half + 1) * S)
            for m in range(2):
                for kf in range(4):
                    nc.tensor.matmul(
                        pmd3s[m][:, sl], lhsT=w1_sb[:, kf, m, :],
                        rhs=xt_sb[:, kf, half, :],
                        start=(kf == 0), stop=(kf == 3),
                        skip_group_check=True)
                nc.vector.tensor_scalar(
                    d3t[:, m, sl], pmd3s[m][:, sl],
                    scalar1=b1c_sb[:, m:m + 1], scalar2=0.0,
                    op0=ALU.add, op1=ALU.max)

        def emit_tv(b):
            # pmtv lives in the pzs rotation (same 2-bank size) so tv
            # matmuls never serialize behind the pmm d3 accumulators.
            xt = xt_sb[:, :, b, :]
            pmtv = pzs.tile([128, 4, C], f32, tag="zs", name=f"pmtv_{b}")
            for sc in range(4):
                for kf in range(4):
                    nc.tensor.matmul(
                        pmtv[:, sc, :],
                        lhsT=xt[:, kf, sc * 128:(sc + 1) * 128],
                        rhs=wv_sb[:, kf, :],
                        start=(kf == 0), stop=(kf == 3))
            tv = tvpool.tile([128, 4, C], f16, tag="tv", name=f"tv{b}")
            nc.scalar.activation(tv[:, :, :], pmtv[:, :, :], AF.Tanh)
            tvs[b] = tv

        def get_us(b, h):
            us = uss.get((b, h))
            if us is None:
                us = usp.tile([128, 2, S], f16, tag="us", name=f"us{b}_{h}")
                uss[(b, h)] = us
            return us

        def emit_zs_pair(p, h):
            # batch-pair N=512 matmuls: consecutive MMs share lhsT (one
            # weight load per (ks, ct) serves both batch halves), so every
            # LDWEIGHTS hides under ~426ns of streaming. (A single N=1024
            # MM is ISA-illegal: matmul output must fit one PSUM bank.)
            blo = 2 * p
            for ct in range(2):
                pzt = pzs.tile([128, 2, S], f32, tag="zs",
                               name=f"pz{p}_{h}_{ct}")
                for ks in range(2):
                    for half in range(2):
                        mm = nc.tensor.matmul(
                            pzt[:, half, :], lhsT=wtop_sb[:, h, ks, ct, :],
                            rhs=d3tps[p][:, ks, half * S:(half + 1) * S],
                            start=(ks == 0), stop=(ks == 1))
                        if half == 1:
                            # weights already resident from the half-0 MM;
                            # skip the redundant LDWEIGHTS (f16 streaming is
                            # 2 elem/cycle, so dup loads serialize the PE).
                            mm.ins.ldweights = False
                for half in range(2):
                    b = blo + half
                    nc.scalar.activation(
                        get_us(b, h)[:, ct, :], pzt[:, half, :],
                        AF.Tanh, bias=zc_sb[:, ct, h, b:b + 1])

        def emit_zs_single(b, h):
            # pair-0 fast path: per-batch N=512 chains off the d3t halves
            half = b % 2
            p = b // 2
            pzt = pzs.tile([128, 2, S], f32, tag="zs", name=f"pzs{b}_{h}")
            for ct in range(2):
                for ks in range(2):
                    nc.tensor.matmul(
                        pzt[:, ct, :], lhsT=wtop_sb[:, h, ks, ct, :],
                        rhs=d3tps[p][:, ks, half * S:(half + 1) * S],
                        start=(ks == 0), stop=(ks == 1))
            for ct in range(2):
                nc.scalar.activation(
                    get_us(b, h)[:, ct, :], pzt[:, ct, :],
                    AF.Tanh, bias=zc_sb[:, ct, h, b:b + 1])

        def emit_pat_quad(b, k):
            # 4 adjacent matmuls covering all 4 col-groups -> concurrent.
            if k == 0:
                pats[b] = patp.tile([128, S], f32, tag="atts",
                                    name=f"pat{b}")
                if b < 2:
                    # zero once per bank; later batches inherit finite
                    # stale values in the never-written rows (masked by
                    # zero comb columns).
                    nc.vector.memset(pats[b][:, :], 0.0)
            pat = pats[b]
            for hh in (2 * k, 2 * k + 1):
                us = uss.pop((b, hh))
                for ct in range(2):
                    g = 2 * (hh % 2) + ct
                    nc.tensor.matmul(pat[32 * g:32 * g + 4, :],
                                     lhsT=pblk_sl(hh * 2 + ct),
                                     rhs=us[:, ct, :],
                                     start=(hh < 2), stop=(hh >= 6),
                                     skip_group_check=True,
                                     tile_position=(0, 32 * g))

        def emit_cast_comb(b):
            attsg = smallsb.tile([128, S], f16, tag="attsg", name=f"ag{b}")
            nc.vector.tensor_copy(out=attsg, in_=pats[b])
            attsgs[b] = attsg
            # pat2 reuses the just-cast pat bank (patp rotation)
            pat2 = patp.tile([8, S], f32, tag="atts", name=f"pat2_{b}")
            nc.tensor.matmul(pat2, lhsT=comb_sb, rhs=attsg,
                             start=True, stop=True)
            nmax = smallsb.tile([8, 1], f32, tag="nmax", name=f"nmax{b}")
            nc.vector.tensor_reduce(nmax, pat2, axis=mybir.AxisListType.X,
                                    op=ALU.max, negate=True)
            pat2s[b] = pat2
            nmaxs[b] = nmax

        def emit_exp(b):
            esc = smallsb.tile([8, S], f16, tag="esc", name=f"esc{b}")
            zsum = smallsb.tile([8, 1], f32, tag="zsum", name=f"zsum{b}")
            nc.scalar.activation(esc, pat2s[b], AF.Exp, bias=nmaxs[b],
                                 accum_out=zsum)
            zinv = smallsb.tile([8, 1], f32, tag="zinv", name=f"zinv{b}")
            nc.vector.reciprocal(zinv, zsum)
            escs[b] = esc
            zinvs[b] = zinv

        def emit_tail_psc(b):
            psc = pmm.tile([128, 4, 8], f16, tag="mm", name=f"psc{b}")
            for sc in range(4):
                nc.tensor.transpose(psc[:, sc, :],
                                    in_=escs[b][:, sc * 128:(sc + 1) * 128],
                                    identity=id8_sb)
            sct = smallsb.tile([128, 4, 8], f16, tag="sct", name=f"sct{b}")
            nc.vector.tensor_copy(out=sct, in_=psc)
            scts[b] = sct

        def emit_tail_vs(b):
            pvs = pmm.tile([8, C], f32, tag="mm", name=f"pvs{b}")
            for sc in range(4):
                nc.tensor.matmul(pvs, lhsT=scts[b][:, sc, :],
                                 rhs=tvs[b][:, sc, :],
                                 start=(sc == 0), stop=(sc == 3))
            vssb = smallsb.tile([8, C], f16, tag="vssb", name=f"vssb{b}")
            nc.vector.tensor_scalar_mul(vssb, pvs, zinvs[b])
            vssbs[b] = vssb

        def emit_tail_pvt(b):
            pvt = pmm.tile([128, 2, 8], f16, tag="mm", name=f"pvt{b}")
            for ch in range(2):
                nc.tensor.transpose(
                    pvt[:, ch, :],
                    in_=vssbs[b][:, ch * 128:(ch + 1) * 128],
                    identity=id8_sb)
            for ch in range(2):
                nc.vector.tensor_copy(out=v_sb[:, ch, :, b:b + 1],
                                      in_=pvt[:, ch, :])

        # ---- pipelined emission ----
        # Pair p head loop interleaves: zs/tanh of pair p, P-reduction
        # quads of both pair batches (even->pat bank A, odd->bank B), and
        # the softmax tail of pair p-1 (exp deferred past tanh(p,1) so the
        # PE->DVE->PE->DVE chain latency hides under ACT work).
        # Startup: b0's d3/zs chains first so the first tanh fires ~13us.
        emit_d3_half(0)
        emit_d4()
        emit_zc(0, 2)
        emit_zs_single(0, 0)
        emit_zs_single(0, 1)
        emit_d3_half(1)
        emit_zc(2, 4)
        emit_zs_single(1, 0)
        emit_zs_single(1, 1)
        emit_zc(4, 8)
        emit_tv(0)
        emit_tv(1)
        for p in range(NPAIR):
            blo, bhi = 2 * p, 2 * p + 1
            plo, phi = blo - 2, bhi - 2  # previous pair's batches
            for h in range(2 if p == 0 else 0, H):
                if p == 0:
                    emit_zs_single(0, h)
                    emit_zs_single(1, h)
                else:
                    emit_zs_pair(p, h)
                if h == 1 and p > 0:
                    emit_exp(plo)
                    emit_exp(phi)
                if h == 2:
                    emit_pat_quad(blo, 0)
                    if p > 0:
                        emit_tail_psc(plo)
                if h == 3:
                    emit_pat_quad(bhi, 0)
                    if p > 0:
                        emit_tail_vs(plo)
                        emit_tail_psc(phi)
                if h == 4:
                    emit_pat_quad(blo, 1)
                    if p > 0:
                        emit_tail_pvt(plo)
                        emit_tail_vs(phi)
                if h == 5:
                    emit_pat_quad(bhi, 1)
                    if p > 0:
                        emit_tail_pvt(phi)
                if h == 6:
                    emit_pat_quad(blo, 2)
                if h == 7:
                    emit_pat_quad(bhi, 2)
            emit_pat_quad(blo, 3)
            emit_pat_quad(bhi, 3)
            emit_cast_comb(blo)
            emit_cast_comb(bhi)
            if p + 1 < NPAIR:
                # d3 m-halves interleaved with tv so the m1 matmuls never
                # wait on m0's relu eviction (pmm is a single buffer).
                d3t = d3p.tile([128, 2, 2 * S], f16, tag="d3t",
                               name=f"d3t{p + 1}")
                d3tps[p + 1] = d3t
                for m in range(2):
                    pmd3 = pmm.tile([128, 2, S], f32, tag="mm",
                                    name=f"pmd3_{p + 1}_{m}")
                    for kf in range(4):
                        for half in range(2):
                            mm = nc.tensor.matmul(
                                pmd3[:, half, :], lhsT=w1_sb[:, kf, m, :],
                                rhs=xt_sb[:, kf, 2 * p + 2 + half, :],
                                start=(kf == 0), stop=(kf == 3))
                            if half == 1:
                                mm.ins.ldweights = False
                    for half in range(2):
                        nc.vector.tensor_scalar(
                            d3t[:, m, half * S:(half + 1) * S],
                            pmd3[:, half, :],
                            scalar1=b1c_sb[:, m:m + 1], scalar2=0.0,
                            op0=ALU.add, op1=ALU.max)
                    emit_tv(2 * p + 2 + m)
        # last pair's tail (no next pair to hide under)
        for b in (BLOC - 2, BLOC - 1):
            emit_exp(b)
            emit_tail_psc(b)
            emit_tail_vs(b)
            emit_tail_pvt(b)

        # ---- final: out = relu(V.T @ wcc + bcc) ----
        pout = pmm.tile([8, OUTF], f32, tag="mm", name="pout")
        kidx = 0
        for h in range(H):
            for ch in range(2):
                nc.tensor.matmul(pout, lhsT=v_sb[:, ch, h, :],
                                 rhs=wcc_sb[:, h * 2 + ch, :],
                                 start=(kidx == 0), stop=False)
                kidx += 1
        nc.tensor.matmul(pout, lhsT=ones18_sb, rhs=bcc_sb,
                         start=False, stop=True)
        outsb = smallsb.tile([8, OUTF], f32, tag="outsb")
        nc.scalar.activation(outsb, pout, AF.Relu)
        nc.sync.dma_start(out=out_d[:, :], in_=outsb)

    nc.compile()
    _dedup_ldweights(nc.m, mybir)
    return nc


def _dedup_ldweights(m, mybir):
    """Remove redundant InstLdweights whose weights AP equals the previous
    PE weight load (the compile pass splits every matmul into LDWEIGHTS +
    non-self-loading MATMUL, even when consecutive matmuls share lhsT).
    f16 streaming runs 2 elem/cycle, so a dup ~93ns LDW serializes against
    a ~107ns N=512 stream. Only sync-free dups are removed, and tracking
    resets on any PE instruction that could clobber the array weights."""
    PE = mybir.EngineType.PE
    safe = {"InstLdweights", "InstMatmult", "InstEventSemaphore",
            "InstDrain", "InstNop", "InstNotify", "InstTensorLoad",
            "InstCompareBranch"}
    total = 0
    for f in m.functions:
        for bb in f.blocks:
            insts = list(bb.instructions)
            out, lastw, removed = [], None, 0
            for i in insts:
                if getattr(i, "engine", None) == PE:
                    tn = type(i).__name__
                    if tn == "InstLdweights":
                        si = i.sync_info
                        empty = si is None or (not si.on_wait
                                               and not si.on_update)
                        k = str(i.ins[0])
                        if empty and k == lastw:
                            removed += 1
                            continue
                        lastw = k
                    elif tn == "InstMatmult":
                        if i.is_transpose:
                            lastw = None
                    elif tn not in safe:
                        lastw = None
                out.append(i)
            if removed:
                bb.instructions = out
                total += removed
    return total


def host_inputs(d1, d2, w1, b1, W, P, wv, wcc, bcc):
    """Host-side sharding + layout prep. Returns in_maps for 8 cores."""
    d1 = np.ascontiguousarray(d1, dtype=np.float32)
    d2 = np.ascontiguousarray(d2, dtype=np.float32)
    w1 = np.ascontiguousarray(w1, dtype=np.float32)
    b1 = np.ascontiguousarray(b1, dtype=np.float32)
    W = np.ascontiguousarray(W, dtype=np.float32)
    P = np.ascontiguousarray(P, dtype=np.float32)
    wv = np.ascontiguousarray(wv, dtype=np.float32)
    wcc = np.ascontiguousarray(wcc, dtype=np.float32)
    bcc = np.ascontiguousarray(bcc, dtype=np.float32)

    w1r = np.ascontiguousarray(
        w1.reshape(4, 128, 2, 128).transpose(1, 0, 2, 3))
    wvr = np.ascontiguousarray(wv.reshape(4, 128, C).transpose(1, 0, 2))
    wtopr = np.ascontiguousarray(
        W[:, :C, :].reshape(H, 2, 128, 2, 128).transpose(2, 0, 1, 3, 4))
    wbotr = np.ascontiguousarray(
        W[:, C:, :].reshape(H, 2, 128, 2, 128).transpose(2, 0, 1, 3, 4))
    # 4-way col-tiled P blocks: head h, half ct -> col-group g=2*(h%2)+ct,
    # output row r=h//2 within the group.
    pblkr = np.zeros((128, 2 * H, 4), np.float32)
    combr = np.zeros((128, H), np.float32)
    for h in range(H):
        r = h // 2
        for ct in range(2):
            g = 2 * (h % 2) + ct
            pblkr[:, h * 2 + ct, r] = P[h, ct * 128:(ct + 1) * 128]
            combr[32 * g + r, h] = 1.0
    wccr = np.ascontiguousarray(
        wcc.reshape(2 * H, 128, OUTF).transpose(1, 0, 2))
    bccr = np.ascontiguousarray(bcc[None, :])
    b1c = np.ascontiguousarray(b1.reshape(2, 128).T)
    id8 = np.eye(8, dtype=np.float32)
    ones18 = np.ones((1, 8), np.float32)

    f16 = np.float16
    packed0 = np.zeros((128, 248), np.float32)
    packed0[:, 0:8] = combr
    packed0[:, 8:72] = pblkr.reshape(128, 64)
    packed0[0:8, 104:112] = id8
    packed0[0:1, 112:120] = ones18
    packed0[0:1, 120:248] = bccr
    shared = dict(w1r=w1r.astype(f16), wvr=wvr.astype(f16),
                  wtopr=wtopr.astype(f16), wbotr=wbotr.astype(f16),
                  wccr=wccr.astype(f16), b1c=b1c)
    in_maps = []
    for core in range(NCORES):
        bs = slice(core * BLOC, (core + 1) * BLOC)
        # xt[p, kf, b, s] = d2[s, bs.start+b, kf*128+p]
        d2c = d2[:, bs, :]  # [S, BLOC, F]
        xtr = np.ascontiguousarray(
            d2c.transpose(2, 1, 0).reshape(4, 128, BLOC, S)
            .transpose(1, 0, 2, 3)).astype(np.float16)
        d1c = d1[bs]  # [BLOC, F]
        d1tr = d1c.T.reshape(4, 128, BLOC).transpose(1, 0, 2)
        packed = packed0.copy()
        packed[:, 72:104] = d1tr.reshape(128, 32)
        in_maps.append(dict(xt=xtr, packed=packed.astype(f16), **shared))
    return in_maps


def kernel(**inputs):
    if "nc" not in _CACHE:
        _CACHE["nc"] = build_nc()
    nc = _CACHE["nc"]
    in_maps = host_inputs(
        d1=inputs["d1"], d2=inputs["d2"], w1=inputs["w1"], b1=inputs["b1"],
        W=inputs["W"], P=inputs["P"], wv=inputs["wv"], wcc=inputs["wcc"],
        bcc=inputs["bcc"])
    from concourse.bass_utils import run_bass_kernel_spmd
    res = run_bass_kernel_spmd(nc, in_maps, core_ids=list(range(NCORES)))
    return np.concatenate([res.results[i]["out"] for i in range(NCORES)],
                          axis=0)
